# revision 29
# baseline (speedup 1.0000x reference)
"""Distributed multi-head attention kernel for one TRN2 chip (8 NeuronCores).

Problem: x[2, 2048, 1024] -> fused QKV proj (16 heads x 64) -> softmax attention
-> output proj, weights packed as in the reference (qkv interleaved [3, h, d]).

Sharding: 2-way data parallel on batch x 4-way tensor parallel on heads.
Core c = (b = c // 4, g = c % 4) gets batch b and heads [4g, 4g+4).
W_qkv column-sharded by head, W_out row-sharded; bf16 ReduceScatter(add) over
each batch group of 4 cores combines the partial output projections.

Layout per core (bf16 matmuls, fp32 PSUM):
  x^T built by regular PE matmuls against a bf16 identity (moving operand),
  streamed per 512-row block and interleaved with the K^T/Q^T/V projections
  and the first attention sweep so ScalarE exp work starts early.
  Attention: per (blk, head-pair p, kc): scores^T via a row-tiled concurrent
  matmul pair (K=64 each), exp on ScalarE (1/8 scale folded, no max needed),
  PV via ones-augmented V matmul (M=65) accumulating over kc.
  Normalize per p-sweep: DVE evac of sums/pvf, one reciprocal_approx_fast,
  gpsimd partition_broadcast, DVE mul.
  Output projection of block b interleaved into block b+1's attention;
  ReduceScatter post-processing (bias add + f32 store) deferred so
  collectives never head-block a compute queue.  Block 3's RS is split in
  two halves to shorten the exposed tail.
"""
import numpy as np

from concourse import mybir, tile, bacc
from concourse.bass_utils import run_bass_kernel_spmd

S = 2048       # sequence length (one batch element per core)
D = 1024       # embed dim
HL = 4         # local heads per core
HD = 64        # head dim
QKVC = 3 * HL * HD   # 768 local qkv columns
VOFF = 2 * HL * HD   # 512: V column offset within the shard
BLK = 512      # s_q / s_k block size
NBLK = S // BLK      # 4
KC = S // 128        # 16 s_k chunks
DC = D // 128        # 8 dmodel chunks
F32 = mybir.dt.float32
BF16 = mybir.dt.bfloat16
EXP = mybir.ActivationFunctionType.Exp
SCALE = 1.0 / np.sqrt(HD)

REPLICA_GROUPS = [[0, 1, 2, 3], [4, 5, 6, 7]]


def build_nc():
    from contextlib import ExitStack

    nc = bacc.Bacc("TRN2", target_bir_lowering=False, debug=False, num_devices=8)
    x_ext = nc.declare_dram_parameter("x", [S, D], F32, isOutput=False)
    wqkv_ext = nc.declare_dram_parameter("wqkv", [D, QKVC], F32, isOutput=False)
    bqkv_ext = nc.declare_dram_parameter("bqkv", [QKVC], F32, isOutput=False)
    wout_ext = nc.declare_dram_parameter("wout", [HL * HD, D], F32, isOutput=False)
    bout_ext = nc.declare_dram_parameter("bout", [D], F32, isOutput=False)
    out_ext = nc.declare_dram_parameter("out", [NBLK * 128, D], F32, isOutput=True)

    with tile.TileContext(nc) as tc, ExitStack() as top:
        # ---- persistent pools ----
        const = top.enter_context(tc.tile_pool(name="const", bufs=1))
        woutp = top.enter_context(tc.tile_pool(name="woutp", bufs=2))
        wq_stage = top.enter_context(tc.tile_pool(name="wq_stage", bufs=3))
        wq_pool = top.enter_context(tc.tile_pool(name="wq", bufs=DC))
        xf_pool = top.enter_context(tc.tile_pool(name="xf", bufs=6))
        xb_pool = top.enter_context(tc.tile_pool(name="xb", bufs=6))
        xT_pool = top.enter_context(tc.tile_pool(name="xT", bufs=DC))
        qkT_pool = top.enter_context(tc.tile_pool(name="qkT", bufs=2))
        v_pool = top.enter_context(tc.tile_pool(name="v", bufs=KC))
        e_pool = top.enter_context(tc.tile_pool(name="e", bufs=4))
        oT_pool = top.enter_context(tc.tile_pool(name="oT", bufs=4))
        pvf_pool = top.enter_context(tc.tile_pool(name="pvf", bufs=4))
        nrm_pool = top.enter_context(tc.tile_pool(name="nrm", bufs=2))
        rb_pool = top.enter_context(tc.tile_pool(name="rbc", bufs=4))
        stage = top.enter_context(tc.tile_pool(name="stage", bufs=4))
        ostage = top.enter_context(tc.tile_pool(name="ostage", bufs=2))
        rs_dram = top.enter_context(tc.tile_pool(name="rs_dram", bufs=12, space="DRAM"))

        sp_ps = top.enter_context(tc.tile_pool(name="sp_ps", bufs=2, space="PSUM"))
        pv_ps = top.enter_context(tc.tile_pool(name="pv_ps", bufs=2, space="PSUM"))
        mi_ps = top.enter_context(tc.tile_pool(name="mi_ps", bufs=2, space="PSUM"))

        # ---- identity first on the gpsimd queue, then W_qkv loads on its
        # SWDGE; x loads on the sync HWDGE queue ----
        ident = const.tile([128, 128], BF16)
        from concourse.masks import make_identity
        make_identity(nc, ident[:, :])
        ones64 = const.tile([128, HD], BF16)
        nc.vector.memset(ones64[:, :], 1.0)
        wrm_e = const.tile([1, 16], F32)
        nc.scalar.activation(wrm_e[:, :], ident[0:1, 0:16], EXP)


        wq_f = []
        for c in range(DC):
            wf = wq_stage.tile([128, QKVC], F32, tag="wq_f32", name="wq_f32")
            nc.gpsimd.dma_start(out=wf[:, :], in_=wqkv_ext[c * 128:(c + 1) * 128, :])
            wq_f.append(wf)

        bqk_sb = const.tile([128, 4], F32)        # per-partition qk bias, col m
        for m in range(4):
            nc.gpsimd.dma_start(out=bqk_sb[:, m:m + 1],
                                in_=bqkv_ext[m * 128:(m + 1) * 128][:, None])
        bv_sb = const.tile([128, HL * HD], F32)   # v bias broadcast across partitions
        nc.gpsimd.dma_start(out=bv_sb[:, :],
                            in_=bqkv_ext[VOFF:QKVC][None, :].to_broadcast((128, HL * HD)))

        # x chunk loads issued up front, split across both HWDGE queues
        xf = []
        for sc in range(KC):
            t = xf_pool.tile([128, D], F32, tag="x_f32", name="xf")
            eng = nc.sync if (sc // 4) % 2 == 0 else nc.scalar
            eng.dma_start(out=t[:, :], in_=x_ext[sc * 128:(sc + 1) * 128, :])
            xf.append(t)

        # rb0's x casts ahead of the wq casts on the DVE queue so the first
        # transposes aren't gated on all of W_qkv landing.
        xb0 = []
        for sc in range(4):
            b = xb_pool.tile([128, D], BF16, tag="x_bf", name="xb")
            nc.vector.tensor_copy(b[:, :], xf[sc][:, :])
            xb0.append(b)

        wq_bf = []
        for c in range(DC):
            wb = wq_pool.tile([128, QKVC], BF16, tag="wq_bf", name="wq_bf")
            nc.vector.tensor_copy(wb[:, :], wq_f[c][:, :])
            wq_bf.append(wb)

        wout_bf = []
        for p in range(2):
            wf = woutp.tile([128, D], F32, tag="wout_f32")
            nc.sync.dma_start(out=wf[:, :], in_=wout_ext[p * 128:(p + 1) * 128, :])
            wb = woutp.tile([128, D], BF16, tag="wout_bf")
            nc.vector.tensor_copy(wb[:, :], wf[:, :])
            wout_bf.append(wb)

        bout_f = const.tile([1, D], F32)
        nc.sync.dma_start(out=bout_f[:, :], in_=bout_ext[None, :])
        bout_full = const.tile([128, D], F32)
        nc.gpsimd.partition_broadcast(bout_full[:, :], bout_f[:, :])
        bout_q = const.tile([128, D], F32)     # b_out/4: added on each of the
        nc.vector.tensor_scalar_mul(bout_q[:, :], bout_full[:, :], 0.25)

        # tiny warmup collective: absorbs the first-collective sync/skew cost
        # while the prologue computes; result unused
        wrm_in = rs_dram.tile([4, 16], BF16, tag="wrm_in", name="wrm_in", bufs=1)
        wrm_out = rs_dram.tile([1, 16], BF16, tag="wrm_out", name="wrm_out", bufs=1)
        nc.gpsimd.collective_compute(
            "ReduceScatter", mybir.AluOpType.add,
            replica_groups=REPLICA_GROUPS,
            ins=[wrm_in[:, :].opt()], outs=[wrm_out[:, :].opt()])

        xT = [xT_pool.tile([128, S], BF16, tag="xT", name="xT") for _ in range(DC)]
        kT = [qkT_pool.tile([128, S], BF16, tag="kT", name="kT", bufs=2)
              for _ in range(2)]
        qT = [[qkT_pool.tile([128, BLK], BF16, tag="qT", name="qT", bufs=8)
               for _ in range(NBLK)] for _ in range(2)]
        v_sb = [v_pool.tile([128, HL * (HD + 1)], BF16, tag="v_sb", name="v_sb")
                for _ in range(KC)]

        def qkv_mm(m, blk):
            ps = mi_ps.tile([128, BLK], F32, tag="mi", name="qkv_ps")
            for c in range(DC):
                nc.tensor.matmul(ps[:, :], wq_bf[c][:, m * 128:(m + 1) * 128],
                                 xT[c][:, blk * BLK:(blk + 1) * BLK],
                                 start=(c == 0), stop=(c == DC - 1))
            return ps

        def k_proj(mk, blk):
            ps = qkv_mm(2 + mk, blk)
            nc.vector.tensor_add(kT[mk][:, blk * BLK:(blk + 1) * BLK], ps[:, :],
                                 bqk_sb[:, 2 + mk:3 + mk].to_broadcast((128, BLK)))

        def q_proj(mq, blk):
            ps = qkv_mm(mq, blk)
            nc.vector.tensor_add(qT[mq][blk][:, :], ps[:, :],
                                 bqk_sb[:, mq:mq + 1].to_broadcast((128, BLK)))

        def v_proj(sc):
            ps = mi_ps.tile([128, HL * HD], F32, tag="mi", name="v_ps")
            for c in range(DC):
                nc.tensor.matmul(ps[:, :], xT[c][:, sc * 128:(sc + 1) * 128],
                                 wq_bf[c][:, VOFF:QKVC],
                                 start=(c == 0), stop=(c == DC - 1))
            vv = v_sb[sc][:, :].rearrange("p (h n) -> p h n", n=HD + 1)
            nc.vector.memset(vv[:, :, HD:HD + 1], 1.0)
            nc.vector.tensor_add(vv[:, :, 0:HD],
                                 ps[:, :].rearrange("p (h d) -> p h d", d=HD),
                                 bv_sb[:, :].rearrange("p (h d) -> p h d", d=HD))

        def stream_rb_a(rb):
            """x chunks of 512-row block rb -> bf16 -> x^T (identity matmuls),
            then K^T (and for rb0, Q^T) columns for this block."""
            if rb == 0:
                xb = xb0
            else:
                xb = []
                for sc in range(4 * rb, 4 * rb + 4):
                    b = xb_pool.tile([128, D], BF16, tag="x_bf", name="xb")
                    nc.vector.tensor_copy(b[:, :], xf[sc][:, :])
                    xb.append(b)
            for c in range(DC):
                tp = mi_ps.tile([128, BLK], F32, tag="mi", name="tp")
                for j in range(4):
                    nc.tensor.matmul(tp[:, j * 128:(j + 1) * 128],
                                     xb[j][:, c * 128:(c + 1) * 128],
                                     ident[:, :], start=True, stop=True)
                # alternate evacuation between DVE and the prologue-idle
                # ScalarE so neither queue gates the x^T pipeline
                if c % 2 == 0:
                    nc.vector.tensor_copy(xT[c][:, rb * BLK:(rb + 1) * BLK],
                                          tp[:, :])
                else:
                    nc.scalar.activation(xT[c][:, rb * BLK:(rb + 1) * BLK],
                                         tp[:, :],
                                         mybir.ActivationFunctionType.Copy)
            for mk in (0, 1):
                k_proj(mk, rb)
            if rb == 0:
                for mq in (0, 1):
                    q_proj(mq, 0)

        def stream_rb_b(rb):
            for sc in range(4 * rb, 4 * rb + 4):
                v_proj(sc)

        # ---- attention helpers ----
        def scores_exp(blk, p, kc):
            ks = slice(kc * 128, (kc + 1) * 128)
            sp = sp_ps.tile([128, 2 * BLK], F32, tag="sp", name="sp")
            nc.tensor.matmul(sp[:, 0:BLK],
                             kT[p][0:64, ks], qT[p][blk][0:64, :],
                             start=True, stop=True)
            nc.tensor.matmul(sp[:, BLK:],
                             kT[p][64:128, ks], qT[p][blk][64:128, :],
                             start=True, stop=True)
            e = e_pool.tile([128, 2 * BLK], BF16, tag="e", name="e")
            nc.scalar.activation(e[:, :], sp[:, :], EXP, scale=float(SCALE))
            return e

        def pv_only(p, kc, e, pvA, pvB):
            nc.tensor.matmul(
                pvA[:, :],
                v_sb[kc][:, (2 * p) * (HD + 1):(2 * p + 1) * (HD + 1)],
                e[:, 0:BLK], start=(kc == 0), stop=(kc == KC - 1),
                skip_group_check=True)
            nc.tensor.matmul(
                pvB[:, :],
                v_sb[kc][:, (2 * p + 1) * (HD + 1):(2 * p + 2) * (HD + 1)],
                e[:, BLK:], start=(kc == 0), stop=(kc == KC - 1),
                skip_group_check=True)

        def attn_iter(blk, p, kc, pvA, pvB):
            e = scores_exp(blk, p, kc)
            pv_only(p, kc, e, pvA, pvB)

        CPY = mybir.ActivationFunctionType.Copy

        def normalize_evac(pvA, pvB, on_act=False):
            """Evacuate the two PV psums of one head pair: denominator rows
            to partitions 0/32 of one tile (cross-partition copies),
            numerators to pvf tiles.  The last sweep uses the post-attention
            idle ScalarE instead of the tail-saturated DVE."""
            sums = nrm_pool.tile([128, BLK], F32, tag="sums", name="sums")
            pvf = []
            for hh, pv in ((0, pvA), (1, pvB)):
                if on_act:
                    nc.scalar.activation(sums[32 * hh:32 * hh + 1, :],
                                         pv[HD:HD + 1, :], CPY)
                else:
                    nc.vector.tensor_copy(sums[32 * hh:32 * hh + 1, :],
                                          pv[HD:HD + 1, :])
                pf = pvf_pool.tile([HD, BLK], F32, tag="pvf", name="pvf")
                if on_act:
                    nc.scalar.activation(pf[:, :], pv[0:HD, :], CPY)
                else:
                    nc.vector.tensor_copy(pf[:, :], pv[0:HD, :])
                pvf.append(pf)
            return sums, pvf

        def normalize_fin(sums, pvf, ot):
            """One full-width reciprocal_approx_fast, rank-1 matmul broadcast
            (keeps gpsimd free for collectives), multiply into ot."""
            rc = nrm_pool.tile([128, BLK], F32, tag="rc", name="rc")
            nc.vector.reciprocal_approx_fast(rc[:, :], sums[:, :])
            rcb = nrm_pool.tile([128, BLK], BF16, tag="rcb", name="rcb")
            nc.scalar.activation(rcb[:, :], rc[:, :], CPY)
            for hh in (0, 1):
                rbp = mi_ps.tile([64, BLK], F32, tag="mi", name="rbp")
                nc.tensor.matmul(rbp[:, :], ones64[32 * hh:32 * hh + 1, :],
                                 rcb[32 * hh:32 * hh + 1, :],
                                 start=True, stop=True)
                nc.vector.tensor_mul(ot[hh * 64:(hh + 1) * 64, :],
                                     pvf[hh][:, :], rbp[:, :])

        def outproj_half(oTb, sq, nh, st):
            po = mi_ps.tile([128, BLK], F32, tag="mi", name="po")
            ns = slice(nh * 512, (nh + 1) * 512)
            nc.tensor.matmul(po[:, :], oTb[0][:, sq * 128:(sq + 1) * 128],
                             wout_bf[0][:, ns], start=True, stop=False)
            nc.tensor.matmul(po[:, :], oTb[1][:, sq * 128:(sq + 1) * 128],
                             wout_bf[1][:, ns], start=False, stop=True)
            nc.vector.tensor_add(st[:, ns], po[:, :], bout_q[:, ns])

        def outproj_sq(oTb, sq, rs_in, row0):
            st = stage.tile([128, D], BF16, tag="st", name="st")
            for nh in range(2):
                outproj_half(oTb, sq, nh, st)
            nc.sync.dma_start(out=rs_in[row0:row0 + 128, :], in_=st[:, :])

        def issue_rs(rs_in, nrows):
            rs_out = rs_dram.tile([nrows // 4, D], BF16, tag=f"rs_out{nrows}",
                                  name="rs_out")
            nc.gpsimd.collective_compute(
                "ReduceScatter", mybir.AluOpType.add,
                replica_groups=REPLICA_GROUPS,
                ins=[rs_in[:, :].opt()], outs=[rs_out[:, :].opt()])
            return rs_out

        def rs_post(rs_out, nrows, out_row0, on_dve=False):
            """Deferred RS post-processing: load + f32 cast + store.  Early
            blocks cast on the idle gpsimd engine (an RS wait there can't
            block compute); the tail blocks cast on DVE (7x faster, and by
            then nothing is left for the wait to block).  Bias was already
            folded in before the RS."""
            ro = ostage.tile([nrows, D], BF16, tag="ro", name="ro")
            nc.sync.dma_start(out=ro[:, :], in_=rs_out[:, :])
            rof = ostage.tile([nrows, D], F32, tag="rof", name="rof")
            if on_dve:
                nc.scalar.activation(rof[:, :], ro[:, :], CPY)
            else:
                nc.gpsimd.tensor_copy(rof[:, :], ro[:, :])
            nc.sync.dma_start(out=out_ext[out_row0:out_row0 + nrows, :],
                              in_=rof[:, :])

        # ================= main schedule =================
        # Streaming prologue: per 512-row block, x^T + K/Q/V projections,
        # interleaved with the first attention sweep (blk0, p=0).
        pvA0 = pv_ps.tile([HD + 1, BLK], F32, tag="pv", name="pv")
        pvB0 = pv_ps.tile([HD + 1, BLK], F32, tag="pv", name="pv")
        for rb in range(NBLK):
            stream_rb_a(rb)
            es = [scores_exp(0, 0, kc) for kc in range(4 * rb, 4 * rb + 4)]
            stream_rb_b(rb)
            for kc, e in zip(range(4 * rb, 4 * rb + 4), es):
                pv_only(0, kc, e, pvA0, pvB0)

        # prev = (oT, rs_in, blk) awaiting output projection into blk+1;
        # pend = (sums, pvf, ot) evacuated sweep awaiting normalize_fin —
        # finished after the NEXT sweep's first two iters so the exp stream
        # never waits on the reciprocal chain
        prev = None
        pend = None
        post_q = []   # deferred RS post-processing: (rs_out, nrows, out_row0)
        oT = []
        for blk in range(NBLK):
            for p in range(2):
                if blk == 0 and p == 0:
                    pvA, pvB = pvA0, pvB0          # computed in the prologue
                else:
                    pvA = pv_ps.tile([HD + 1, BLK], F32, tag="pv", name="pv")
                    pvB = pv_ps.tile([HD + 1, BLK], F32, tag="pv", name="pv")
                    e_prev = None
                    for kc in range(KC):
                        # PV trails scores by one iteration: the interleave
                        # bursts below run while ScalarE computes this exp,
                        # and PV(kc-1)'s input is always ready
                        e_cur = scores_exp(blk, p, kc)
                        if kc == 1 and pend is not None:
                            normalize_fin(*pend)
                            pend = None
                        # interleave prev block's output projection + RS issue
                        if p == 0 and prev is not None:
                            if kc in (3, 4, 6, 7, 9, 10, 12, 13):
                                bi = {3: 0, 4: 1, 6: 2, 7: 3, 9: 4, 10: 5,
                                      12: 6, 13: 7}[kc]
                                sq, nh = bi // 2, bi % 2
                                if nh == 0:
                                    st_cur = stage.tile([128, D], BF16,
                                                        tag="st", name="st")
                                outproj_half(prev[0], sq, nh, st_cur)
                                if nh == 1:
                                    nc.sync.dma_start(
                                        out=prev[1][sq * 128:(sq + 1) * 128, :],
                                        in_=st_cur[:, :])
                            elif kc == 14:
                                post_q.append((issue_rs(prev[1], 4 * 128),
                                               128, prev[2] * 128))
                                prev = None
                        elif p == 1 and blk + 1 < NBLK:
                            if kc == 5:
                                q_proj(0, blk + 1)
                            elif kc == 10:
                                q_proj(1, blk + 1)
                        if e_prev is not None:
                            pv_only(p, kc - 1, e_prev, pvA, pvB)
                        e_prev = e_cur
                    pv_only(p, KC - 1, e_prev, pvA, pvB)
                sums, pvf = normalize_evac(
                    pvA, pvB, on_act=(blk == NBLK - 1 and p == 1))
                ot = oT_pool.tile([128, BLK], BF16, tag="ot", name="ot")
                pend = (sums, pvf, ot)
                oT.append(ot)
            if blk < NBLK - 1:
                rs_in = rs_dram.tile([4 * 128, D], BF16, tag="rs_in", name="rs_in")
                prev = (oT[-2:], rs_in, blk)
        # tail: finish blk3 p1's normalize, project, one full RS
        normalize_fin(*pend)
        rs_in = rs_dram.tile([4 * 128, D], BF16, tag="rs_in", name="rs_in")
        for sq in range(4):
            outproj_sq(oT[-2:], sq, rs_in, sq * 128)
        post_q.append((issue_rs(rs_in, 4 * 128), 128, 3 * 128))
        for qi, args in enumerate(post_q):
            rs_post(*args, on_dve=(qi >= len(post_q) - 2))

    nc.compile()
    return nc


_NC = None


def kernel(x, W_qkv, b_qkv, W_out, b_out):
    global _NC
    if _NC is None:
        _NC = build_nc()

    cols = np.concatenate([np.arange(t * 1024, t * 1024 + 256) for t in range(3)])
    in_maps = []
    for c in range(8):
        b, g = c // 4, c % 4
        gcols = cols + g * 256
        in_maps.append({
            "x": np.ascontiguousarray(x[b]),
            "wqkv": np.ascontiguousarray(W_qkv[:, gcols]),
            "bqkv": np.ascontiguousarray(b_qkv[gcols]),
            "wout": np.ascontiguousarray(W_out[g * 256:(g + 1) * 256, :]),
            "bout": np.ascontiguousarray(b_out),
        })

    res = run_bass_kernel_spmd(_NC, in_maps, core_ids=list(range(8)))

    out = np.empty((2, S, D), np.float32)
    for c in range(8):
        b, g = c // 4, c % 4
        r = res.results[c]["out"]
        for k in range(NBLK):
            out[b, k * BLK + g * 128: k * BLK + (g + 1) * 128, :] = \
                r[k * 128:(k + 1) * 128, :]
    return out
